# revision 14
# baseline (speedup 1.0000x reference)
"""CoordEncoder Trainium2 kernel, v2.

Data-parallel over B across 8 NeuronCores (one batch element per core).
Per core, for its L=1024 atoms (i-tiles of 128 rows, j free over all 1024):
  q[i,j] = |x_i - x_j|^2 via one K=5 augmented matmul on PE (aug matrices
  packed on host).
  d = sqrt(q + eps) in one ACT op (reads PSUM directly).
  16 RBF planes f_r = exp(-gamma*(d - c_r)^2) in bf16:
    - seed r=0:  exp(-gamma*q) on ACT straight from PSUM
    - seed r=6:  Square(d - c6) + Exp on ACT
    - seed r=12: (q - 2*c12*d) on DVE scalar_tensor_tensor, Exp on ACT
    - the rest chained with one bf16 tensor_tensor mult per plane
      (f~_r = f~_{r-1} * ts, ts = exp(2*gamma*dc*d + TSB)); the resulting
      constant per-plane drift is divided out of Wg on the host.
      Chains run on DVE (2x bf16) except GPLANES which go to GpSimd.
  Reductions over the neighbor axis j:
    - ACT-seeded planes: free accum_out row-sums on the ACT op -> rs[128,16]
    - DVE_RED planes: 4x-mode tensor_scalar pass with accum_out -> rs
    - all other planes: PE ones-column matmuls (col sums == row sums by
      symmetry) accumulated over i-tiles into a persistent cs[16,1024] PSUM
  rs is transposed per-tile on PE (identity matmul); the bf16 tail per tile
  is three N=256 matmuls into one PSUM: onehot(Z)^T @ t1 + rs^T @ Wg +
  cs-slice @ Wg.  Every plane's sum lives in exactly one of rs/cs (the
  other holds zeros), so the two Wg matmuls sum to the exact total.

Host side only packs layouts and folds input-independent weights.
"""

import numpy as np

B, L, E, R, NA = 8, 1024, 256, 16, 118
P = 128          # partition tile
NT = L // P      # 8 i-tiles per core
SEEDS = (0, 6, 12)      # planes seeded directly on ACT
GPLANES = (5, 11, 15)   # chained planes computed on GpSimd instead of DVE
DVE_RED = ()            # chained planes reduced via DVE tensor_scalar pass
Q_F32R = False          # q matmul in float32r (full-rate) instead of fp32
TSB = -5.0           # shared bias inside ts = exp(2*gamma*dc*d + TSB)
EPS = 1e-4           # sqrt(q + EPS); covers fp32 matmul cancellation error

_CACHE = {}


def _seg_of(r):
    s = max(x for x in SEEDS if x <= r)
    return s, r - s


def _build_nc(gamma, centers, split=True):
    import concourse.bass as bass
    import concourse.tile as tile
    from concourse import mybir
    from contextlib import ExitStack

    f32 = mybir.dt.float32
    bf16 = mybir.dt.bfloat16
    AF = mybir.ActivationFunctionType
    ALU = mybir.AluOpType

    dc = float(centers[1] - centers[0])
    c6, c12 = float(centers[6]), float(centers[12])
    PERED = [r for r in range(R) if r not in SEEDS and r not in DVE_RED]

    nc = bass.Bass("TRN2", target_bir_lowering=False, debug=False)

    lhsa = nc.dram_tensor("lhsa", [5, L], f32, kind="ExternalInput")
    rhsa = nc.dram_tensor("rhsa", [5, L], f32, kind="ExternalInput")
    t1 = nc.dram_tensor("t1", [NA, E], bf16, kind="ExternalInput")
    wg = nc.dram_tensor("wg", [R, E], bf16, kind="ExternalInput")
    oneh = nc.dram_tensor("oneh", [NA, L], bf16, kind="ExternalInput")
    ident = nc.dram_tensor("ident", [P, P], f32, kind="ExternalInput")
    onesc = nc.dram_tensor("onesc", [P, R * R], bf16, kind="ExternalInput")
    out = nc.dram_tensor("out", [L, E], f32, kind="ExternalOutput")

    with tile.TileContext(nc) as tc, ExitStack() as ctx:
        consts = ctx.enter_context(tc.tile_pool(name="consts", bufs=1))

        # ---- persistent SBUF tensors (all DMA'd straight from host) ----
        lhs_aug = consts.tile([5, L], f32)      # [-2x,-2y,-2z, 1, |x|^2]
        rhs_aug = consts.tile([5, L], f32)      # [x, y, z, |x|^2, 1]
        t1s = consts.tile([NA, E], bf16)
        wgs = consts.tile([R, E], bf16)
        onehotT = consts.tile([NA, L], bf16)    # onehot(Z)^T
        idn = consts.tile([P, P], f32)
        onescol = consts.tile([P, R, R], bf16)  # one-hot cols for PE colsums
        nc.sync.dma_start(lhs_aug[:], lhsa[:, :])
        nc.scalar.dma_start(rhs_aug[:], rhsa[:, :])
        nc.scalar.dma_start(onescol[:], onesc[:, :])
        nc.sync.dma_start(idn[:], ident[:, :])
        nc.scalar.dma_start(wgs[:], wg[:, :])
        nc.sync.dma_start(t1s[:], t1[:, :])
        nc.sync.dma_start(onehotT[:], oneh[:, :])

        # per-partition bias constants for activation ops
        eps_b = consts.tile([P, 1], f32, tag="eps_b")
        nc.vector.memset(eps_b[:], EPS)
        tsb = consts.tile([P, 1], f32, tag="tsb")
        nc.vector.memset(tsb[:], TSB)
        nc6 = consts.tile([P, 1], f32, tag="nc6")
        nc.vector.memset(nc6[:], -c6)
        g12b = consts.tile([P, 1], f32, tag="g12b")
        nc.vector.memset(g12b[:], -gamma * c12 * c12)

        # ---- pools ----
        from contextlib import ExitStack as _ES
        loop_ctx = _ES()
        cspp = ctx.enter_context(tc.tile_pool(name="cs_ps", bufs=1, space="PSUM"))
        qpp = loop_ctx.enter_context(tc.tile_pool(name="q_ps", bufs=2, space="PSUM"))
        rtpp = loop_ctx.enter_context(tc.tile_pool(name="rt_ps", bufs=2, space="PSUM"))
        ddp = ctx.enter_context(tc.tile_pool(name="dd", bufs=3))
        ttp = ctx.enter_context(tc.tile_pool(name="tt", bufs=3))
        sqp = ctx.enter_context(tc.tile_pool(name="sq", bufs=3))
        fbp = ctx.enter_context(tc.tile_pool(name="fb", bufs=20))
        trp = ctx.enter_context(tc.tile_pool(name="tr", bufs=2))
        rsp = ctx.enter_context(tc.tile_pool(name="rs", bufs=4))
        rtsp = ctx.enter_context(tc.tile_pool(name="rts", bufs=1))
        csbp = ctx.enter_context(tc.tile_pool(name="csb", bufs=1))
        hbp = ctx.enter_context(tc.tile_pool(name="hb", bufs=3))

        if Q_F32R:
            f32r = mybir.dt.float32r
            lhs_r = consts.tile([5, L], f32r, tag="lhs_r")
            rhs_r = consts.tile([5, L], f32r, tag="rhs_r")
            nc.vector.tensor_copy(lhs_r[:], lhs_aug[:])
            nc.vector.tensor_copy(rhs_r[:], rhs_aug[:])
            lhs_q, rhs_q = lhs_r, rhs_r
        else:
            lhs_q, rhs_q = lhs_aug, rhs_aug

        cs = cspp.tile([R, L], f32)     # PE-reduced plane sums (col sums)
        rsT = []                        # per-tile transposed row sums (SBUF)

        # PE warmup: ramp the tensor engine during the input DMAs so the
        # first real matmuls run at full clock (cold-PE fp32 is ~3x slower).
        wz = consts.tile([1, 512], bf16, tag="wz")
        nc.vector.memset(wz[:], 0.0)
        for w in range(10):
            nc.tensor.matmul(cs[0:1, 0:512], wz[0:1, 0:1], wz[:, :],
                             start=True, stop=True, skip_group_check=True)

        n_colsum_total = NT * len(PERED)
        colsum_state = {"n": 0}
        colsum_q = []

        def emit_colsums():
            while colsum_q:
                r, f = colsum_q.pop(0)
                for h in range(2):
                    nc.tensor.matmul(cs[:, h * 512:(h + 1) * 512],
                                     onescol[:, r, :],
                                     f[:, h * 512:(h + 1) * 512],
                                     start=(colsum_state["n"] == 0),
                                     stop=(colsum_state["n"] == n_colsum_total - 1 and h == 1),
                                     skip_group_check=True)
                colsum_state["n"] += 1

        for it in range(NT):
            i0 = it * P
            # pairwise squared distances for this row block: [128, 1024]
            qps = qpp.tile([P, L], f32)
            for h in range(2):
                nc.tensor.matmul(qps[:, h * 512:(h + 1) * 512],
                                 lhs_q[:, i0:i0 + P],
                                 rhs_q[:, h * 512:(h + 1) * 512],
                                 start=True, stop=True)
            emit_colsums()
            rs = rsp.tile([P, R], f32)
            nc.vector.memset(rs[:], 0.0)

            dd = ddp.tile([P, L], f32)
            nc.scalar.activation(dd[:], qps[:], AF.Sqrt, bias=eps_b[:])
            ts = ttp.tile([P, L], bf16)
            nc.scalar.activation(ts[:], dd[:], AF.Exp,
                                 scale=2.0 * gamma * dc, bias=tsb[:])

            planes = {}
            # seed 0: exp(-gamma*q) straight from PSUM
            f0 = fbp.tile([P, L], bf16, tag="plane")
            nc.scalar.activation(f0[:], qps[:], AF.Exp, scale=-gamma,
                                 accum_out=rs[:, 0:1])
            planes[0] = f0
            # seed 6: Square(d - c6) then Exp
            sq6 = sqp.tile([P, L], f32)
            nc.scalar.activation(sq6[:], dd[:], AF.Square, bias=nc6[:])
            f6 = fbp.tile([P, L], bf16, tag="plane")
            nc.scalar.activation(f6[:], sq6[:], AF.Exp, scale=-gamma,
                                 accum_out=rs[:, 6:7])
            planes[6] = f6
            # seed 12: (q - 2*c12*d) on DVE, then Exp(-gamma*x - gamma*c12^2)
            sh12 = sqp.tile([P, L], f32, tag="sh12")
            nc.vector.scalar_tensor_tensor(sh12[:], dd[:], -2.0 * c12, qps[:],
                                           op0=ALU.mult, op1=ALU.add)
            f12 = fbp.tile([P, L], bf16, tag="plane")
            nc.scalar.activation(f12[:], sh12[:], AF.Exp, scale=-gamma,
                                 bias=g12b[:], accum_out=rs[:, 12:13])
            planes[12] = f12

            # chains
            for s in SEEDS:
                f = planes[s]
                end = min(s + 6, R)
                for r in range(s + 1, end):
                    fn = fbp.tile([P, L], bf16, tag="plane")
                    # last tile: keep the slow GpSimd chains off the drain path
                    eng = nc.gpsimd if (r in GPLANES and it < NT - 1) else nc.vector
                    eng.tensor_tensor(fn[:], f[:], ts[:], ALU.mult)
                    if r in DVE_RED:
                        trash = trp.tile([P, L], bf16)
                        nc.vector.tensor_scalar(trash[:], fn[:], 1.0, None,
                                                ALU.mult, accum_out=rs[:, r:r + 1])
                    planes[r] = fn
                    f = fn

            # reductions: DVE 4x pass for DVE_RED planes
            for r in DVE_RED:
                trash = trp.tile([P, L], bf16)
                nc.vector.tensor_scalar(trash[:], planes[r][:], 1.0, None,
                                        ALU.mult, accum_out=rs[:, r:r + 1])
            # PE ones-column col sums (everything not ACT- or DVE-reduced) for the rest, accumulated across tiles
            for k, r in enumerate(PERED):
                f = planes[r]
                for h in range(2):
                    nc.tensor.matmul(cs[:, h * 512:(h + 1) * 512],
                                     onescol[:, r, :],
                                     f[:, h * 512:(h + 1) * 512],
                                     start=(it == 0 and k == 0),
                                     stop=(it == NT - 1 and k == len(PERED) - 1),
                                     skip_group_check=True)

            # transpose rs -> [16, 128] and stash in SBUF (bf16) for the tail
            rtp = rtpp.tile([R, P], f32)
            nc.tensor.transpose(rtp[:], rs[:], idn[:])
            rts = rtsp.tile([R, P], bf16, tag=f"rts{it}")
            nc.scalar.copy(rts[:], rtp[:])
            rsT.append(rts)

        emit_colsums()

        # ---- tail: project and emit ----
        loop_ctx.close()
        csb = csbp.tile([R, L], bf16)
        with tc.tile_pool(name="h_ps", bufs=2, space="PSUM") as hpp:
            for it in range(NT):
                i0 = it * P
                nc.vector.tensor_copy(csb[:, i0:i0 + P], cs[:, i0:i0 + P])
                hps = hpp.tile([P, E], f32)
                nc.tensor.matmul(hps[:], onehotT[:, i0:i0 + P], t1s[:],
                                 start=True, stop=False, skip_group_check=True)
                nc.tensor.matmul(hps[:], rsT[it][:], wgs[:],
                                 start=False, stop=False, skip_group_check=True)
                nc.tensor.matmul(hps[:], csb[:, i0:i0 + P], wgs[:],
                                 start=False, stop=True, skip_group_check=True)
                hb = hbp.tile([P, E], f32)
                nc.scalar.copy(hb[:], hps[:])
                nc.sync.dma_start(out[i0:i0 + P, :], hb[:])

    if split:
        _split_excess_waits(nc)
    return nc


def _split_excess_waits(nc, maxw=1):
    """This walrus build rejects instructions carrying more than one sem wait
    (setupSyncWait: 'Too many sync wait commands'). Move excess waits onto
    injected same-engine NOPs that execute immediately before."""
    from concourse import mybir
    n = 0
    for fn in nc.m.functions:
        for bb in fn.blocks:
            new = []
            for ins in bb.instructions:
                si = ins.sync_info
                if si is not None and si.on_wait and len(si.on_wait) > maxw:
                    waits = list(si.on_wait)
                    excess, keep = waits[:-maxw], waits[-maxw:]
                    for ci in range(0, len(excess), maxw):
                        nop = mybir.InstNoOp(name=f"waitsplit_{ins.name}_{ci}",
                                             ins=[], outs=[])
                        nop.engine = ins.engine
                        nop.bass_nofuse = True
                        nop.sync_info = mybir.SyncInfo(on_wait=excess[ci:ci + maxw],
                                                       on_update=[])
                        new.append(nop)
                        n += 1
                    si.on_wait = keep
                new.append(ins)
            bb.instructions[:] = new
    return n


def _np_bf16():
    from concourse import mybir
    return mybir.dt.np(mybir.dt.bfloat16)


def _prep_inputs(coords, Z, atom_emb, rbf_centers, gamma, rbf_proj_w,
                 rbf_proj_b, out_proj_w, out_proj_b):
    f64 = np.float64
    bf = _np_bf16()
    g = float(np.asarray(gamma))
    centers = np.asarray(rbf_centers, dtype=f64)
    dc = float(centers[1] - centers[0])
    w1 = np.asarray(out_proj_w)[:E].astype(f64)
    w2 = np.asarray(out_proj_w)[E:].astype(f64)
    bias = (np.asarray(rbf_proj_b).astype(f64) @ w2) + np.asarray(out_proj_b).astype(f64)
    t1 = (np.asarray(atom_emb).astype(f64) @ w1 + bias).astype(bf)
    wgm = (np.asarray(rbf_proj_w).astype(f64) @ w2) / L
    # divide out the chain drift from using the shared ts (see _build_nc)
    for r in range(R):
        s, m = _seg_of(r)
        if m:
            wgm[r] /= np.exp(m * TSB + g * dc * (2 * centers[s] * m + dc * m * m))
    wgm = wgm.astype(bf)
    ident = np.eye(P, dtype=np.float32)
    onesc = np.zeros((P, R * R), dtype=np.float64)
    for r in range(R):
        onesc[:, r * R + r] = 1.0
    onesc = onesc.astype(bf)
    cf = np.asarray(coords, dtype=f64)                     # [B, L, 3]
    nsq = (cf * cf).sum(-1)                                # [B, L]
    ones = np.ones((L,), dtype=f64)
    Zl = np.asarray(Z)
    in_maps = []
    for b in range(B):
        xT = cf[b].T                                       # [3, L]
        lhs = np.concatenate([-2.0 * xT, ones[None, :], nsq[b][None, :]], axis=0)
        rhs = np.concatenate([xT, nsq[b][None, :], ones[None, :]], axis=0)
        onehotT = np.zeros((NA, L), dtype=np.float64)
        onehotT[Zl[b], np.arange(L)] = 1.0
        in_maps.append({
            "lhsa": lhs.astype(np.float32),
            "rhsa": rhs.astype(np.float32),
            "t1": t1, "wg": wgm,
            "oneh": onehotT.astype(bf),
            "ident": ident, "onesc": onesc,
        })
    return in_maps


def _get_nc(gamma, centers):
    key = (float(gamma),) + tuple(float(c) for c in centers)
    if key not in _CACHE:
        _CACHE[key] = _build_nc(float(gamma), [float(c) for c in centers])
    return _CACHE[key]


def _run(in_maps, gamma, centers, trace=False):
    from concourse.bass_utils import run_bass_kernel_spmd
    nc = _get_nc(gamma, centers)
    return run_bass_kernel_spmd(nc, in_maps, core_ids=list(range(B)), trace=trace)


def kernel(coords, Z, atom_emb, rbf_centers, gamma, rbf_proj_w, rbf_proj_b,
           out_proj_w, out_proj_b):
    centers = np.asarray(rbf_centers, dtype=np.float64)
    steps = np.diff(centers)
    assert np.allclose(steps, steps[0], rtol=1e-5), "uniform RBF grid expected"
    in_maps = _prep_inputs(coords, Z, atom_emb, rbf_centers, gamma, rbf_proj_w,
                           rbf_proj_b, out_proj_w, out_proj_b)
    res = _run(in_maps, float(np.asarray(gamma)), centers)
    return np.stack([res.results[b]["out"] for b in range(B)], axis=0)
